# revision 34
# baseline (speedup 1.0000x reference)
"""Local (sliding-window) causal attention kernel for Trainium2, 8 NeuronCores.

Problem: nn_LocalAttention (B=2, S=2048, D=1024, nh=16, hd=64, window=256,
topk=0).  q = x @ Wq.T ; k,v = reshaped inputs ; scores masked to the strict
causal band  qi-256 <= kj <= qi-1 ; softmax ; out = (P @ v) heads concat @ Wo.T.

Sharding: data-parallel over (B, S): 8 shards of 512 query rows; each core gets
its key/value halo of 768 rows.  No collectives.

Device layout: everything is computed in "transposed" (feature-major) layout so
no on-device transposes are needed:
  - host passes xT [D, 512], kT [D, 768], Wq.T, Wo.T; v stays natural.
  - qT = WqT.T @ xT                        (PE)
  - ST[kj, qi] = kT_h.T @ qT_h             (PE, banded windows only)
  - ST = exp(ST/8) * bandmask              (ACT + DVE; no max needed, scores~N(0,1))
  - attnT_unnorm[hd, qi], den[qi] = [v_h | 1].T @ ST   (PE, ones-column trick,
      misaligned windows accumulate via PSUM has_written semantics)
  - norm: dens broadcast across partitions via tiny K=1 PE outer product, one
      reciprocal_approx on the pair tile, fused into the PSUM->SBUF copy
  - outT = WoT.T @ attnT_norm              (PE) ; host transposes back.

Matmul inputs are bf16 (PE fp32 moving-operand throughput is ~4x lower), all
accumulation in fp32 PSUM; softmax denominators exact in fp32 up to the bf16
rounding of the broadcast.  Set DTYPE="f32" for the full-precision variant.
"""

import os
import numpy as np

DTYPE = os.environ.get("LA_DTYPE", "bf16")

NCORES = 8
B, S, D = 2, 2048, 1024
NH, HD = 16, 64
ROWS = 512            # query rows per core
HALO = 256            # window size
KROWS = ROWS + HALO   # 768 key rows per core
NKJ = KROWS // 128    # 6 key chunks

# qi-window of each kj-chunk cj: all qi chunks that the band of cj touches.
WIN = [(max(0, 128 * (cj - 2)), min(ROWS, 128 * cj + 128)) for cj in range(NKJ)]
WIDTHS = [hi - lo for lo, hi in WIN]
MOFF = np.concatenate([[0], np.cumsum(WIDTHS)]).astype(int)  # mask col offsets
MTOT = int(MOFF[-1])  # 1536

_prog = None  # cached compiled program


def _build_program(reps=1, phases=(1, 2, 3), loop_n=0):
    """reps>1: python-unrolled reps (differential timing).  loop_n>0: wrap the
    body in a hardware For_i loop with that trip count (low-noise timing;
    program size independent of loop_n)."""
    from contextlib import ExitStack
    import concourse.tile as tile
    from concourse import bacc, mybir

    f32 = mybir.dt.float32
    DT = mybir.dt.bfloat16 if DTYPE == "bf16" else f32
    nc = bacc.Bacc("TRN2", target_bir_lowering=False, debug=False,
                   enable_asserts=False)

    # register an eps const AP (only 0.0/1.0 are pre-registered) for the
    # denominator guard: recip(0) is undefined in reciprocal_approx_fast.
    EPS = 1e-20
    eps_t = nc.alloc_sbuf_tensor("const-eps", [128, 1], f32)
    nc.gpsimd.memset(eps_t.ap(), EPS)
    nc.const_aps.aps[(f32, EPS)] = eps_t.ap()
    nc.all_engine_barrier()

    d_xT = nc.dram_tensor("xT", [D, ROWS], DT, kind="ExternalInput").ap()
    d_kT = nc.dram_tensor("kT", [D, KROWS], DT, kind="ExternalInput").ap()
    d_va = nc.dram_tensor("va", [KROWS, NH * 65], DT, kind="ExternalInput").ap()
    d_wq = nc.dram_tensor("wqT", [D, D], DT, kind="ExternalInput").ap()
    d_wo = nc.dram_tensor("woT", [D, D], DT, kind="ExternalInput").ap()
    d_msk = nc.dram_tensor("msk", [128, MTOT], DT, kind="ExternalInput").ap()
    d_out = nc.dram_tensor("outT", [D, ROWS], DT, kind="ExternalOutput").ap()

    EXP = mybir.ActivationFunctionType.Exp

    with tile.TileContext(nc) as tc, ExitStack() as ctx:
        pers = ctx.enter_context(tc.tile_pool(name="pers", bufs=1))
        PS_MM = int(os.environ.get("LA_PSMM", "1"))
        PS_ST = int(os.environ.get("LA_PSST", "4"))
        PS_AV = int(os.environ.get("LA_PSAV", "2"))
        ps_mm = ctx.enter_context(
            tc.tile_pool(name="psmm", bufs=PS_MM, space="PSUM"))
        ps_st = ctx.enter_context(
            tc.tile_pool(name="psst", bufs=PS_ST, space="PSUM"))
        ps_av = ctx.enter_context(
            tc.tile_pool(name="psav", bufs=PS_AV, space="PSUM"))
        st_pool = ctx.enter_context(tc.tile_pool(name="stp", bufs=8))
        kt_pool = ctx.enter_context(tc.tile_pool(name="ktp", bufs=1))
        bc_pool = ctx.enter_context(tc.tile_pool(name="bcp", bufs=2))
        ot_pool = ctx.enter_context(tc.tile_pool(name="otp", bufs=2))
        den_pool = ctx.enter_context(tc.tile_pool(name="denp", bufs=4))
        wqx = ctx.enter_context(tc.tile_pool(name="wqx", bufs=1))

        def body(rep=0):
            par = rep % 2   # parity double-buffering of cross-phase tensors
            QCOPY = os.environ.get("LA_QCOPY", "dve")
            # ---- phase 1 inputs first: wq/x gate the q projection, which
            # gates everything else; remaining loads issue behind them.
            wq_t, x_t = [], []
            xdma = nc.scalar if int(os.environ.get("LA_RING", "1")) \
                else nc.sync
            for k2 in range(8):
                t = wqx.tile([128, ROWS], DT, tag=f"x{k2}_{par}", name=f"x{k2}")
                xdma.dma_start(out=t[:],
                               in_=d_xT[128 * k2:128 * k2 + 128, :])
                x_t.append(t)
            for k2 in range(8):
                t = wqx.tile([128, D], DT, tag=f"wq{k2}_{par}", name=f"wq{k2}")
                nc.sync.dma_start(out=t[:], in_=d_wq[128 * k2:128 * k2 + 128, :])
                wq_t.append(t)

            ones128 = pers.tile([128, 64], DT, tag="ones128")
            nc.vector.memset(ones128[:], 1.0)
            attnT = [pers.tile([128, ROWS], DT, tag=f"at{p}_{par}", name=f"at{p}")
                     for p in range(8)]
            qT_t = []

            # ---- phase 1: q projection ----
            for m in range(8):
                q = pers.tile([128, ROWS], DT, tag=f"qT{m}_{par}", name=f"qT{m}")
                if 1 in phases:
                    ps = ps_mm.tile([128, ROWS], f32, tag=f"mm{par}", name="ps_mm_t")
                    for k2 in range(8):
                        nc.tensor.matmul(ps[:],
                                         wq_t[k2][:, 128 * m:128 * m + 128],
                                         x_t[k2][:], start=(k2 == 0),
                                         stop=(k2 == 7))
                    if QCOPY == "dve":
                        nc.vector.tensor_copy(q[:], ps[:])
                    else:
                        nc.scalar.copy(out=q[:], in_=ps[:])
                else:
                    nc.vector.memset(q[:], 0.01)
                qT_t.append(q)

            # ---- remaining persistent loads, in order of first use ----
            kt_t = {}
            for p in range(8):
                kt_t[p] = kt_pool.tile([128, KROWS], DT,
                                       tag=f"kt{p}_{par}", name=f"kt{p}")
                nc.sync.dma_start(out=kt_t[p][:],
                                  in_=d_kT[128 * p:128 * p + 128, :])
            msk_t = pers.tile([128, MTOT], DT, tag="msk")
            xdma.dma_start(out=msk_t[:], in_=d_msk[:, :])
            va_t = []
            for cj in range(NKJ):
                t = pers.tile([128, NH * 65], DT, tag=f"va{cj}", name=f"va{cj}")
                xdma.dma_start(out=t[:],
                               in_=d_va[128 * cj:128 * cj + 128, :])
                va_t.append(t)
            wo_t = []  # wo needed last
            for t2 in range(8):
                t = pers.tile([128, D], DT, tag=f"wo{t2}", name=f"wo{t2}")
                nc.sync.dma_start(out=t[:],
                                  in_=d_wo[128 * t2:128 * t2 + 128, :])
                wo_t.append(t)

            # ---- phase 2: attention, head pairs processed in groups of 2 ----
            # ST chunk tiles are packed [c0|c1], [c2], [c3], [c4|c5] (each
            # [128,384], one PSUM bank) -> 4 exp + 4 mask-mul per sub.  The
            # sub0/sub1 ST matmuls are interleaved so consecutive matmuls use
            # different PE row groups (rows 0-63 vs 64-127), letting the PE
            # pull LDWEIGHTS of one ahead of the other's streaming (2.3x).
            # GRP[i] = (chunks, col offset within packed tile, width)
            GRP = [((0, 1), (0, 128), 384), ((2,), (0,), 384),
                   ((3,), (0,), 384), ((4, 5), (0, 256), 384)]
            CJ_GI = {0: (0, 0), 1: (0, 1), 2: (1, 0), 3: (2, 0),
                     4: (3, 0), 5: (3, 1)}
            NP = 8 if 2 in phases else 0

            BC = os.environ.get("LA_BC", "pe")
            DENE = os.environ.get("LA_DEN", "dve")

            def norm_stage(p, av_pair):
                if BC.startswith("pb"):
                    # dens at rows 0/32 (f32), reciprocal once, then GPSIMD
                    # partition-broadcast of the reciprocals (no PE, no PSUM).
                    dg = den_pool.tile([33, ROWS], f32, tag="den",
                                       name="den_g")
                    for sub in range(2):
                        eng = nc.scalar if DENE == "act" else nc.vector
                        if DENE == "act":
                            nc.scalar.add(dg[32 * sub:32 * sub + 1, :],
                                          av_pair[sub][64:65, :], EPS)
                        else:
                            nc.vector.tensor_scalar_add(
                                dg[32 * sub:32 * sub + 1, :],
                                av_pair[sub][64:65, :], EPS)
                    rg = den_pool.tile([33, ROWS], f32, tag="rg", name="rg_g")
                    nc.vector.reciprocal_approx_fast(out=rg[:], in_=dg[:])
                    bc_sb = bc_pool.tile([128, ROWS], f32, tag="bc",
                                         name="bc_sb")
                    pbe = {"pb": nc.gpsimd, "pbsp": nc.sync,
                           "pbdve": nc.vector}[BC]
                    pbe.partition_broadcast(bc_sb[0:64, :], rg[0:1, :],
                                            channels=64)
                    pbe.partition_broadcast(bc_sb[64:128, :], rg[32:33, :],
                                            channels=64)
                else:
                    # dens at partitions 0/32 of one tile, then 2
                    # PE-concurrent K=1 broadcast matmuls via tile_position.
                    dg = den_pool.tile([33, ROWS], DT, tag="den",
                                       name="den_g")
                    for sub in range(2):
                        use_act = DENE == "act" or (DENE == "split"
                                                    and sub == 0)
                        if use_act:
                            nc.scalar.add(dg[32 * sub:32 * sub + 1, :],
                                          av_pair[sub][64:65, :], EPS)
                        else:
                            nc.vector.tensor_scalar_add(
                                dg[32 * sub:32 * sub + 1, :],
                                av_pair[sub][64:65, :], EPS)
                    bc_ps = ps_mm.tile([128, ROWS], f32, tag=f"mm{par}",
                                       name="ps_mm_t")
                    for sub in range(2):
                        i = 32 * sub
                        nc.tensor.matmul(
                            bc_ps[64 * sub:64 * sub + 64, :],
                            ones128[i:i + 1, :], dg[i:i + 1, :],
                            start=True, stop=True, skip_group_check=True,
                            tile_position=(i, 64 * sub))
                    bc_sb = bc_pool.tile([128, ROWS], f32, tag="bc",
                                         name="bc_sb")
                    nc.vector.reciprocal_approx_fast(out=bc_sb[:],
                                                     in_=bc_ps[:])
                for sub in range(2):
                    nc.vector.tensor_mul(
                        attnT[p][64 * sub:64 * sub + 64, :],
                        av_pair[sub][0:64, :],
                        bc_sb[64 * sub:64 * sub + 64, :])

            # software-pipelined pair loop: normalization of pair p-1 is
            # emitted during pair p so no engine queue head-of-line blocks
            # on a cross-engine round trip; kt is prefetched one pair ahead.
            PIPE = int(os.environ.get("LA_PIPE", "0"))
            GPM = int(os.environ.get("LA_GPM", "2"))
            prev = None
            for p in range(NP):                                  # head pair
                kt = kt_t.pop(p)
                qt = qT_t[p]
                # ST matmuls: both subs share one [128,1024] PSUM tile (2
                # banks; cols 0:384 sub0, 384:768 sub1, 768:1024 pad), so one
                # exp covers both subs.  Sub-alternating matmul order keeps
                # consecutive matmuls on different PE row groups (rows 0-63
                # vs 64-127), letting LDWEIGHTS overlap streaming (2.3x).
                # per (group, sub) one [128,384] PSUM tile (single bank);
                # sub-alternating matmul order keeps consecutive matmuls on
                # different PE row groups so LDWEIGHTS overlaps streaming.
                ss_m = [[], []]                # per sub: 4 packed ss tiles
                for gi, (cjs, offs, w) in enumerate(GRP):
                    sps = [ps_st.tile([128, w], f32, tag="stp", name="sp_st")
                           for _ in range(2)]
                    for ci, cj in enumerate(cjs):
                        lo, hi = WIN[cj]
                        for sub in range(2):
                            b0 = 64 * sub
                            nc.tensor.matmul(
                                sps[sub][:, offs[ci]:offs[ci] + hi - lo],
                                kt[b0:b0 + 64, 128 * cj:128 * cj + 128],
                                qt[b0:b0 + 64, lo:hi],
                                start=(ci == 0), stop=(ci == len(cjs) - 1),
                                skip_group_check=True)
                    m0 = int(MOFF[cjs[0]])
                    for sub in range(2):
                        ss = st_pool.tile([128, w], DT, tag="st", name="ss_st")
                        nc.scalar.activation(ss[:], sps[sub][:], EXP,
                                             scale=0.125)
                        eng = nc.gpsimd if (GPM == 2 or (GPM == 1 and (gi + sub + p) % 2 == 0)) \
                            else nc.vector
                        eng.tensor_mul(ss[:], ss[:], msk_t[:, m0:m0 + w])
                        ss_m[sub].append(ss)

                def av_stage(p, ss_m):
                    av_pair = []
                    for sub in range(2):
                        h = 2 * p + sub
                        av = ps_av.tile([65, ROWS], f32, tag="av",
                                        name="av_ps")
                        for cj in range(NKJ):
                            lo, hi = WIN[cj]
                            gi, ci = CJ_GI[cj]
                            off = GRP[gi][1][ci]
                            nc.tensor.matmul(
                                av[:, lo:hi],
                                va_t[cj][:, 65 * h:65 * h + 65],
                                ss_m[sub][gi][:, off:off + hi - lo],
                                start=(cj == 0), stop=(cj == NKJ - 1),
                                skip_group_check=True)
                        av_pair.append(av)
                    return av_pair

                if PIPE == 2:
                    if prev is not None:
                        norm_stage(prev[0], av_stage(*prev))
                    prev = (p, ss_m)
                elif PIPE == 1:
                    av_pair = av_stage(p, ss_m)
                    if prev is not None:
                        norm_stage(*prev)
                    prev = (p, av_pair)
                else:
                    norm_stage(p, av_stage(p, ss_m))
            if prev is not None:
                if PIPE == 2:
                    norm_stage(prev[0], av_stage(*prev))
                else:
                    norm_stage(*prev)

            if 2 not in phases:
                for p2x in range(8):
                    nc.vector.memset(attnT[p2x][:], 0.01)
            # ---- phase 3: output projection ----
            for n in range((8 if 3 in phases else 0)):
                ps = ps_mm.tile([128, ROWS], f32, tag=f"mm{par}", name="ps_mm_t")
                for t2 in range(8):
                    nc.tensor.matmul(ps[:], wo_t[t2][:, 128 * n:128 * n + 128],
                                     attnT[t2][:], start=(t2 == 0), stop=(t2 == 7))
                ot = ot_pool.tile([128, ROWS], DT, tag=f"ot{par}", name="ot_sb")
                if os.environ.get("LA_OCOPY", "act") == "act":
                    nc.scalar.copy(out=ot[:], in_=ps[:])
                else:
                    nc.vector.tensor_copy(ot[:], ps[:])
                nc.sync.dma_start(out=d_out[128 * n:128 * n + 128, :], in_=ot[:])

        if loop_n > 0:
            with tc.For_i(0, loop_n, staggered_reset=bool(
                    int(os.environ.get("LA_STAGGER", "0")))):
                for rep in range(reps):
                    body(rep)
        else:
            for rep in range(reps):
                body(rep)

    nc.compile()
    return nc


def _to_dt(a):
    if DTYPE == "bf16":
        import ml_dtypes
        return np.ascontiguousarray(a).astype(ml_dtypes.bfloat16)
    return np.ascontiguousarray(a).astype(np.float32)


def _host_prep(query_seq, keys_seq, values_seq, Wq, Wo):
    """Build the 8 per-core input maps."""
    qT_all = np.ascontiguousarray(query_seq.transpose(0, 2, 1))  # [B, D, S]
    kT_all = np.ascontiguousarray(keys_seq.transpose(0, 2, 1))
    wqT = _to_dt(Wq.T)
    woT = _to_dt(Wo.T)

    def band_mask(first):
        m = np.zeros((128, MTOT), np.float32)
        for cj in range(NKJ):
            lo, hi = WIN[cj]
            kj = 128 * cj + np.arange(128)[:, None]
            qi = np.arange(lo, hi)[None, :]
            valid = (kj >= qi) & (kj <= qi + HALO - 1)
            if first:
                valid &= (kj >= HALO)
            m[:, MOFF[cj]:MOFF[cj + 1]] = valid.astype(np.float32)
        return m

    msk_first = _to_dt(band_mask(True))
    msk_rest = _to_dt(band_mask(False))

    in_maps = []
    for c in range(NCORES):
        b, ch = c // 4, c % 4
        r0 = ch * ROWS
        xT = _to_dt(qT_all[b][:, r0:r0 + ROWS])
        kT = np.zeros((D, KROWS), np.float32)
        va = np.zeros((KROWS, NH * 65), np.float32)
        va[:, 64::65] = 1.0  # ones column per head
        if ch == 0:
            kT[:, HALO:] = kT_all[b][:, 0:ROWS]
            v_halo = values_seq[b, 0:ROWS]
            va[HALO:, :] = np.concatenate(
                [v_halo.reshape(ROWS, NH, HD),
                 np.ones((ROWS, NH, 1), np.float32)], axis=2).reshape(ROWS, -1)
        else:
            kT[:, :] = kT_all[b][:, r0 - HALO:r0 + ROWS]
            v_halo = values_seq[b, r0 - HALO:r0 + ROWS]
            va[:, :] = np.concatenate(
                [v_halo.reshape(KROWS, NH, HD),
                 np.ones((KROWS, NH, 1), np.float32)], axis=2).reshape(KROWS, -1)
        in_maps.append({
            "xT": xT, "kT": _to_dt(kT), "va": _to_dt(va), "wqT": wqT,
            "woT": woT, "msk": msk_first if ch == 0 else msk_rest,
        })
    return in_maps


def _run(inputs, trace=False):
    global _prog
    from concourse.bass_utils import run_bass_kernel_spmd

    query_seq = np.asarray(inputs["query_seq"], np.float32)
    keys_seq = np.asarray(inputs["keys_seq"], np.float32)
    values_seq = np.asarray(inputs["values_seq"], np.float32)
    Wq = np.asarray(inputs["Wq"], np.float32)
    Wo = np.asarray(inputs["Wo"], np.float32)
    assert int(inputs.get("window", HALO)) == HALO
    assert int(inputs.get("topk", 0)) == 0

    if _prog is None:
        _prog = _build_program()

    in_maps = _host_prep(query_seq, keys_seq, values_seq, Wq, Wo)
    res = run_bass_kernel_spmd(_prog, in_maps, list(range(NCORES)), trace=trace)

    out = np.empty((B, S, D), np.float32)
    for c in range(NCORES):
        b, ch = c // 4, c % 4
        r0 = ch * ROWS
        out[b, r0:r0 + ROWS, :] = res.results[c]["outT"].T
    return out, res


def kernel(**inputs):
    out, _ = _run(inputs)
    return out

